# revision 39
# baseline (speedup 1.0000x reference)
"""BertSelfAttention on 8 Trainium2 NeuronCores (Bass/Tile).

Sharding: data-parallel over batch (B=2) x tensor-parallel over heads
(16 heads -> 4 groups of 4). Core c handles batch c//4, head group c%4,
holding column shards of Wq/Wk/Wv. No collectives.

v2: full bf16 datapath. The host pre-casts x^T and the weight shards to
bf16 (halves HBM traffic, guarantees 1-cycle/row matmul moving rate; the
old f32r moving operands ran at ~half rate). PSUM accumulation stays f32
and the raw output path is f32. Measured end-to-end rel err ~4e-3.

Engine budget per core (S=2048, 4 heads, d=64):
  TensorE: QK proj 65.5k cyc + V proj 41.6k + scores 65.5k (the hh
           score pairs run CONCURRENTLY in PE row groups 0-63/64-127)
           + ctx 131k ~= 304k cyc @ 2.4 GHz ~= 127 us
  ScalarE: 128 exp ACTs on [128,1024] tiles at ~1.10us each ~= 141 us
           -> the saturated engine in steady state
  DVE:     all PSUM evacuations (QK/V/ctx) ~= 21 us
The schedule keeps ACT saturated: attention runs as 8 fine passes
(head-pair p x q-chunk j of 512); emission order per step is
scores(i) -> exp(i) -> ctx(i-1) -> filler, so the TE never blocks on
the same step's exp; V tiles (just-in-time for pass 0) and the m=1
projection sweeps fill TE slack during ACT-bound stretches. Dummy
warm-up matmuls before the first projection hold the PE at 2.4GHz
through the DMA-gated load phase (it otherwise sits at 1.2GHz), and
the projection->attention handoff evacuations are split across
DVE + ACT so no single engine serializes the transition. Weights are
host-pre-arranged into the partition-major SBUF image: DMA with 4KB
per-partition segments runs ~10x faster than the 512B-segment gather
loads a DRAM-side rearrange would produce.

PSUM (8 banks, exact fit):
  tag "ssc" 2x[128,1024] (4): score tiles / K-m0 proj accumulators
  tag "ctx" 2x[65,512]   (2): ctx+denominator accums / Q-m0 accums
  tag "a"   2x[128,512]  (2): V-proj psum + QK-m1 sweeps / Q-m0 accums

Per head the ctx stationary is [V_h | ones] (65 cols): PSUM row 65 of
each ctx tile accumulates the softmax denominators for free. Host
unshards: out[b, :, g*256 + 64h + r] = (ctx_h / sums_h).T
"""

import sys

sys.path.insert(0, "/opt/trn_rl_repo")

import numpy as np

try:
    import ml_dtypes

    _BF16 = ml_dtypes.bfloat16
except ImportError:  # pragma: no cover
    import jax.numpy as jnp

    _BF16 = jnp.bfloat16

import concourse.bass as bass
import concourse.mybir as mybir
import concourse.tile as tile
from concourse import bacc
from concourse import bass_utils as _bass_utils
from concourse.bass_utils import run_bass_kernel_spmd

# NOTE: unlike the f32r baseline, bf16 matmuls are pre-split by the tile
# legalizer into standalone LDWEIGHTS + MATMUL pairs (LDW of tile i+1 is
# emitted right after MATMUL i, giving ping-pong weight-buffer overlap
# natively). walrus --enable-ldw-opt rejects pre-split LDWs, so the stock
# --enable-ldw-opt=false pipeline is kept as-is.

F32 = mybir.dt.float32
BF16 = mybir.dt.bfloat16

HIDDEN = 1024
NUM_HEADS = 16
HEAD = 64
B, S = 2, 2048
N_CORES = 8
GROUPS = 4                      # head groups (tensor parallel)
HG = NUM_HEADS // GROUPS        # heads per group = 4
DG = HG * HEAD                  # 256 cols per group
KT_TILES = HIDDEN // 128        # 8 contraction tiles for projections
ST_TILES = S // 128             # 16 sequence tiles
QC = 512                        # q chunk width (one pass = one chunk)
N_QC = S // QC                  # 4
VAUG = HG * (HEAD + 1)          # 260: [V_h | ones] per head


def _build_kernel():
    nc = bacc.Bacc("TRN2")

    xT = nc.dram_tensor("xT", [HIDDEN, S], BF16, kind="ExternalInput")
    # weights pre-arranged on host into the exact SBUF layout (partition
    # major) so each DMA is a contiguous per-partition memcpy with 4KB
    # descriptors — gather-style rearranged loads (512B descriptors) run
    # ~10x slower. wqk[p, m, kt, :] = [Wq_m | Wk_m][kt*128+p, :]; the m=0
    # half loads before the xT chunks, m=1 after, all on the SP queue (two
    # queues writing one SBUF tile wedges the device).
    wqk = nc.dram_tensor(
        "wqk", [128, 2, KT_TILES, DG], BF16, kind="ExternalInput"
    )
    # wv pre-augmented (per head 64 cols + zero col) and pre-arranged:
    # wv[p, kt, :] = wv_aug[kt*128+p, :]
    wv = nc.dram_tensor(
        "wv", [128, KT_TILES, VAUG], BF16, kind="ExternalInput"
    )
    # per-partition bias cols: bq[0:128], bq[128:], bk[0:128], bk[128:]
    bqk = nc.dram_tensor("bqk", [128, 4], F32, kind="ExternalInput")
    # bv interleaved with 1.0 at each head's ones column [1, 260]
    bv_aug = nc.dram_tensor("bv_aug", [1, VAUG], BF16, kind="ExternalInput")
    out_raw = nc.dram_tensor("out_raw", [VAUG, S], F32, kind="ExternalOutput")

    with tile.TileContext(nc) as tc:
        with (
            tc.tile_pool(name="consts", bufs=1) as consts,
            tc.tile_pool(name="esp", bufs=3) as esp,
            tc.tile_pool(name="outp", bufs=4) as outp,
            tc.tile_pool(name="ps", bufs=2, space="PSUM") as ps,
        ):
            # ---- load inputs (host already cast to bf16: plain HWDGE) ----
            # two parallel HWDGE queues: xT chunks stream on the SP queue
            # while weights + biases go on the Activation queue (idle until
            # the first exp at ~17us). This roughly halves the load phase
            # and keeps the PE p-state ramped (no per-chunk stalls).
            bqk_sb = consts.tile([128, 4], F32)
            nc.scalar.dma_start(bqk_sb[:], bqk[:])
            bvaug_sb = consts.tile([1, VAUG], BF16)
            nc.scalar.dma_start(bvaug_sb[:], bv_aug[:])
            ones_sb = consts.tile([1, QC], BF16)
            nc.vector.memset(ones_sb[:], 1.0)

            # critical chain strictly ordered on the SP queue: the m=0 half
            # of the weights (host packs wqk as [Qm0|Km0|Qm1|Km1]), then the
            # xT chunks in kt order. The m=1 half + wv ride the scalar queue
            # (needed only ~10us later).
            wqk_sb = consts.tile([128, 2, KT_TILES, DG], BF16)
            xT_sb = consts.tile([128, KT_TILES, S], BF16)
            xT_r = xT.rearrange("(ko p) s -> p ko s", p=128)
            wqk0_sb = wqk_sb[:, 0]
            wqk1_sb = wqk_sb[:, 1]
            wv_sb = consts.tile([128, KT_TILES, VAUG], BF16)
            nc.sync.dma_start(wqk0_sb[:], wqk[:, 0])
            for kt in range(KT_TILES):
                nc.sync.dma_start(xT_sb[:, kt, :], xT_r[:, kt, :])
            nc.sync.dma_start(wv_sb[:], wv[:])
            nc.sync.dma_start(wqk1_sb[:], wqk[:, 1])

            QT_sb = consts.tile([128, 2, S], BF16)
            KT_sb = consts.tile([128, 2, S], BF16)
            v_sb = consts.tile([128, ST_TILES, VAUG], BF16)

            # ---- phase 1: m=0 projections, gated by the xT chunk DMAs ----
            # Q-m0 accumulators from tags "ctx"+"a", K-m0 from tag "ssc"
            # (2 sc halves per [128,1024] tile) -> all 8 banks, freed for
            # attention by the evacuations below.
            psq = [
                ps.tile([128, QC], F32, tag="ctx", name="psq0"),
                ps.tile([128, QC], F32, tag="ctx", name="psq1"),
                ps.tile([128, QC], F32, tag="a", name="psq2"),
                ps.tile([128, QC], F32, tag="a", name="psq3"),
            ]
            psk2 = [
                ps.tile([128, 2 * QC], F32, tag="ssc", name=f"psk{t}")
                for t in range(2)
            ]
            psk = [psk2[sc // 2][:, (sc % 2) * QC:(sc % 2 + 1) * QC]
                   for sc in range(N_QC)]
            # warm-up matmuls: the PE p-state governor holds the array at
            # 1.2GHz until it has seen sustained work; these run during the
            # ~5us wait for the first xT chunk so the projections start at
            # 2.4GHz. psq[0] garbage is erased by the group's start=True.
            for _ in range(9):
                nc.tensor.matmul(
                    psq[0][:], ones_sb[0:1, 0:128], ones_sb[0:1, :],
                    start=True, stop=True,
                )
            for kt in range(KT_TILES):
                st, sp = (kt == 0), (kt == KT_TILES - 1)
                for sc in range(N_QC):
                    nc.tensor.matmul(
                        psq[sc][:],
                        wqk0_sb[:, kt, 0:128],
                        xT_sb[:, kt, sc * QC:(sc + 1) * QC],
                        start=st, stop=sp,
                    )
                for sc in range(N_QC):
                    nc.tensor.matmul(
                        psk[sc][:],
                        wqk0_sb[:, kt, 128:256],
                        xT_sb[:, kt, sc * QC:(sc + 1) * QC],
                        start=st, stop=sp,
                    )
            # evacuate (+bias) spread across three engines so the chains run
            # in parallel and attention isn't gated behind one serial queue:
            # "a"-slot tiles first on DVE (V-proj WARs them), K on ACT
            # (scores need KT; identity shares the exp table), Q on gpsimd
            # (ctx tiles WAR psq0/1).
            nc.vector.tensor_scalar_add(
                QT_sb[:, 0, 2 * QC:3 * QC], psq[2][:], bqk_sb[:, 0:1]
            )
            nc.vector.tensor_scalar_add(
                QT_sb[:, 0, 3 * QC:4 * QC], psq[3][:], bqk_sb[:, 0:1]
            )
            for sc in range(N_QC):
                nc.scalar.activation(
                    KT_sb[:, 0, sc * QC:(sc + 1) * QC], psk[sc][:],
                    mybir.ActivationFunctionType.Identity,
                    bias=bqk_sb[:, 2:3],
                )
            nc.vector.tensor_scalar_add(
                QT_sb[:, 0, 0:QC], psq[0][:], bqk_sb[:, 0:1]
            )
            nc.vector.tensor_scalar_add(
                QT_sb[:, 0, QC:2 * QC], psq[1][:], bqk_sb[:, 0:1]
            )

            # ---- helpers ----
            def v_proj(st):
                psv = ps.tile([128, QC], F32, tag="a", name="psv")
                for kt in range(KT_TILES):
                    nc.tensor.matmul(
                        psv[:, 0:VAUG],
                        xT_sb[:, kt, st * 128:(st + 1) * 128],
                        wv_sb[:, kt, :],
                        start=(kt == 0), stop=False,
                    )
                # bias (and the per-head ones columns) as a rank-1 update
                nc.tensor.matmul(
                    psv[:, 0:VAUG], ones_sb[:, 0:128], bvaug_sb[:, :],
                    start=False, stop=True,
                )
                nc.vector.tensor_copy(out=v_sb[:, st, :], in_=psv[:, 0:VAUG])

            # m=1 projection sweep units: 4 filler units of 2 matmuls per
            # (dst, sc); the last unit finishes the accumulation + evacuates.
            def qk_m1_unit(dst_sb, wcol, bcol, sc, quarter):
                tag_name = f"m1_{wcol}_{sc}"
                if quarter == 0:
                    acc = ps.tile([128, QC], F32, tag="a", name="m1acc")
                    _m1_acc[tag_name] = acc
                else:
                    acc = _m1_acc[tag_name]
                for kt in range(quarter * 2, quarter * 2 + 2):
                    nc.tensor.matmul(
                        acc[:],
                        wqk1_sb[:, kt, wcol:wcol + 128],
                        xT_sb[:, kt, sc * QC:(sc + 1) * QC],
                        start=(kt == 0), stop=(kt == KT_TILES - 1),
                    )
                if quarter == 3:
                    del _m1_acc[tag_name]
                    nc.vector.tensor_scalar_add(
                        dst_sb[:, 1, sc * QC:(sc + 1) * QC], acc[:],
                        bqk_sb[:, bcol:bcol + 1],
                    )

            _m1_acc = {}

            # filler queue: V tiles 2..15 just-in-time for the first pass,
            # then the m=1 Q/K projection sweeps (must finish before the
            # p=1 passes start).
            fillers = []
            for st in range(2, ST_TILES):
                fillers.append(lambda st=st: v_proj(st))
            for sc in range(N_QC):
                for dst, wcol, bcol in ((QT_sb, 0, 1), (KT_sb, 128, 3)):
                    for quarter in range(4):
                        fillers.append(
                            lambda d=dst, w=wcol, b=bcol, s=sc, q=quarter:
                            qk_m1_unit(d, w, b, s, q)
                        )

            # head start for the first pass
            v_proj(0)
            v_proj(1)

            # ---- phase 2: 8 attention passes, software-pipelined ----
            # iteration i handles (pass, kt); emission order per i:
            #   scores(i) -> exp(i) -> ctx(i-1) -> filler
            # so the TE never blocks on the ACT result of the same kt.
            passes = [(p, j) for p in range(2) for j in range(N_QC)]
            steps = [(pi, kt) for pi in range(len(passes))
                     for kt in range(ST_TILES)]
            n_steps = len(steps)

            es_tiles = {}
            ctx_tiles = {}
            fill_budget = 0.0
            # filler pacing: V tiles (~9 matmuls) during pass 0, m=1 units
            # (4 matmuls) later; target spreading them across the ACT-bound
            # iterations without starving the V just-in-time deadline.
            def emit_scores(pi, kt):
                p, j = passes[pi]
                ssc = ps.tile([128, 2 * QC], F32, tag="ssc", name="ssc")
                for hh in range(2):
                    rows = slice(hh * 64, hh * 64 + 64)
                    nc.tensor.matmul(
                        ssc[:, hh * QC:(hh + 1) * QC],
                        KT_sb[rows, p, kt * 128:(kt + 1) * 128],
                        QT_sb[rows, p, j * QC:(j + 1) * QC],
                        start=True, stop=True,
                    )
                es = esp.tile([128, 2 * QC], BF16, tag="es", name="es")
                nc.scalar.activation(
                    es[:], ssc[:], mybir.ActivationFunctionType.Exp,
                    scale=0.125,
                )
                es_tiles[(pi, kt)] = es

            def emit_ctx(pi, kt):
                p, j = passes[pi]
                if kt == 0:
                    for hh in range(2):
                        ctx_tiles[(pi, hh)] = ps.tile(
                            [65, QC], F32, tag="ctx", name="ctx"
                        )
                es = es_tiles.pop((pi, kt))
                for hh in range(2):
                    h = 2 * p + hh
                    nc.tensor.matmul(
                        ctx_tiles[(pi, hh)][:],
                        v_sb[:, kt, h * 65:(h + 1) * 65],
                        es[:, hh * QC:(hh + 1) * QC],
                        start=(kt == 0), stop=(kt == ST_TILES - 1),
                    )
                if kt == ST_TILES - 1:
                    for hh in range(2):
                        h = 2 * p + hh
                        ctx_sb = outp.tile([65, QC], F32, tag="o",
                                           name="ctx_sb")
                        nc.vector.tensor_copy(
                            out=ctx_sb[:], in_=ctx_tiles.pop((pi, hh))[:]
                        )
                        nc.sync.dma_start(
                            out_raw[h * 65:(h + 1) * 65,
                                    j * QC:(j + 1) * QC],
                            ctx_sb[:],
                        )

            # V[kt] for pass 0 is consumed at iteration kt; keep production
            # >= 2 tiles ahead there. After pass 0 spread the rest evenly:
            # ~30 remaining filler units over ~96 ACT-bound iterations, but
            # the m=1 sweeps must all land before step 64 (pass (1,0)).
            filler_deadline = {}
            for idx in range(14):           # V tiles 2..15
                filler_deadline[idx] = idx  # emit at step idx at the latest
            for u in range(14, 14 + 32):    # m=1 units, even ~1.5-step cadence
                filler_deadline[u] = 16 + ((u - 14) * 3) // 2

            fq = list(range(len(fillers)))
            for i, (pi, kt) in enumerate(steps):
                emit_scores(pi, kt)
                if i > 0:
                    emit_ctx(*steps[i - 1])
                while fq and filler_deadline[fq[0]] <= i:
                    fillers[fq.pop(0)]()
            emit_ctx(*steps[n_steps - 1])
            while fq:
                fillers[fq.pop(0)]()
    nc.compile()
    return nc


_NC_CACHE = None


def _get_nc():
    global _NC_CACHE
    if _NC_CACHE is None:
        _NC_CACHE = _build_kernel()
    return _NC_CACHE


def _prep_core_inputs(hidden_states, Wq, bq, Wk, bk, Wv, bv):
    """Host-side sharding: returns list of 8 in_maps (bf16 pre-cast)."""
    xTs = [
        np.ascontiguousarray(hidden_states[b].T).astype(_BF16)
        for b in range(B)
    ]
    in_maps = []
    for c in range(N_CORES):
        b, g = divmod(c, GROUPS)
        cs = slice(g * DG, (g + 1) * DG)
        wq_g = Wq[:, cs]
        wk_g = Wk[:, cs]
        wv_g = Wv[:, cs]
        bq_g, bk_g, bv_g = bq[cs], bk[cs], bv[cs]

        wv_aug = np.zeros((HIDDEN, VAUG), dtype=np.float32)
        bv_aug = np.zeros((1, VAUG), dtype=np.float32)
        for h in range(HG):
            wv_aug[:, h * 65:h * 65 + 64] = wv_g[:, h * 64:(h + 1) * 64]
            bv_aug[0, h * 65:h * 65 + 64] = bv_g[h * 64:(h + 1) * 64]
            bv_aug[0, h * 65 + 64] = 1.0

        bqk = np.stack(
            [bq_g[:128], bq_g[128:], bk_g[:128], bk_g[128:]], axis=1
        ).astype(np.float32)

        in_maps.append(
            {
                "xT": xTs[b],
                # partition-major SBUF image [128, 2, 8, 256]
                "wqk": np.ascontiguousarray(
                    np.stack([
                        np.concatenate(
                            [wq_g[:, m * 128:(m + 1) * 128],
                             wk_g[:, m * 128:(m + 1) * 128]], 1
                        ).reshape(KT_TILES, 128, DG).transpose(1, 0, 2)
                        for m in range(2)
                    ], axis=1)
                ).astype(_BF16),
                # partition-major SBUF image [128, 8, 260]
                "wv": np.ascontiguousarray(
                    wv_aug.reshape(KT_TILES, 128, VAUG).transpose(1, 0, 2)
                ).astype(_BF16),
                "bqk": np.ascontiguousarray(bqk),
                "bv_aug": bv_aug.astype(_BF16),
            }
        )
    return in_maps


def _unshard(results):
    out = np.empty((B, S, HIDDEN), dtype=np.float32)
    for c in range(N_CORES):
        b, g = divmod(c, GROUPS)
        raw = results[c]["out_raw"]  # [260, 2048]
        for h in range(HG):
            ctx = raw[h * 65:h * 65 + 64]          # [64, S]
            sums = raw[h * 65 + 64]                # [S]
            col0 = g * DG + h * HEAD
            out[b, :, col0:col0 + HEAD] = (ctx / sums).T
    return out


def kernel(**inputs):
    inputs = {k: np.asarray(v, dtype=np.float32) for k, v in inputs.items()}
    nc = _get_nc()
    in_maps = _prep_core_inputs(**inputs)
    res = run_bass_kernel_spmd(nc, in_maps, core_ids=list(range(N_CORES)))
    return _unshard(res.results)


if __name__ == "__main__":
    rng = np.random.default_rng(0)
    scale = 1.0 / np.sqrt(HIDDEN)
    ins = {
        "hidden_states": rng.standard_normal((B, S, HIDDEN), dtype=np.float32),
        "Wq": rng.standard_normal((HIDDEN, HIDDEN), dtype=np.float32) * scale,
        "bq": rng.standard_normal(HIDDEN, dtype=np.float32) * 0.01,
        "Wk": rng.standard_normal((HIDDEN, HIDDEN), dtype=np.float32) * scale,
        "bk": rng.standard_normal(HIDDEN, dtype=np.float32) * 0.01,
        "Wv": rng.standard_normal((HIDDEN, HIDDEN), dtype=np.float32) * scale,
        "bv": rng.standard_normal(HIDDEN, dtype=np.float32) * 0.01,
    }
    out = kernel(**ins)

    def ref(x, Wq, bq, Wk, bk, Wv, bv):
        q = (x @ Wq + bq).reshape(B, S, NUM_HEADS, HEAD).transpose(0, 2, 1, 3)
        k = (x @ Wk + bk).reshape(B, S, NUM_HEADS, HEAD).transpose(0, 2, 1, 3)
        v = (x @ Wv + bv).reshape(B, S, NUM_HEADS, HEAD).transpose(0, 2, 1, 3)
        s = np.einsum("bhqd,bhkd->bhqk", q, k) / np.sqrt(HEAD)
        s = s - s.max(-1, keepdims=True)
        p = np.exp(s)
        p /= p.sum(-1, keepdims=True)
        c = np.einsum("bhqk,bhkd->bhqd", p, v)
        return c.transpose(0, 2, 1, 3).reshape(B, S, HIDDEN)

    exp = ref(
        ins["hidden_states"].astype(np.float64),
        ins["Wq"].astype(np.float64), ins["bq"].astype(np.float64),
        ins["Wk"].astype(np.float64), ins["bk"].astype(np.float64),
        ins["Wv"].astype(np.float64), ins["bv"].astype(np.float64),
    )
    print("L2 rel err:", np.linalg.norm(out - exp) / np.linalg.norm(exp))
    print("max abs err:", np.abs(out - exp).max())
